# revision 39
# baseline (speedup 1.0000x reference)
"""Multi-head causal attention (B=4, T=2048, E=1024, H=16, D=64) on 8 trn2
NeuronCores via Bass/Tile.

Sharding: core c handles batch b = c//2 and heads [half*8, half*8+8), half =
c%2. Each core computes its 8 heads' attention and a partial output
projection Y^T = Wp_slice^T-contraction over its heads; the host sums the two
half partials per batch, transposes, and adds the bias.

On-device layout is "transposed": activations are [feature, token] so every
matmul contracts over the partition dim. Softmax denominators come from a
ones-column appended to the stationary V operand (M=65 matmuls); masking is
applied block-wise (128x128) in place on the exp'd scores so each AV tile is
a single long matmul. The t-tiles are processed in descending order with a
filler queue (projections / output-projection groups) interleaved between
attention ivals: the attention inner loop is ACT(exp)-bound, so the PE gets
dense matmul filler while exp runs. No max-subtraction is needed: scores are
~N(0, 0.083^2).
"""
import numpy as np
import ml_dtypes
from collections import deque
from contextlib import ExitStack

import concourse.bass as bass
import concourse.mybir as mybir
import concourse.tile as tile
from concourse.bass_utils import run_bass_kernel_spmd
from concourse.vector_clock import ScopedClock

BF16 = mybir.dt.bfloat16
F32 = mybir.dt.float32
NPBF16 = ml_dtypes.bfloat16

B, T, E, H, D = 4, 2048, 1024, 16, 64
HPC = 8            # heads per core
DC = HPC * D       # 512: stacked head dim per core
TJ = 512           # t tile (matmul free dim)
NJ = T // TJ       # 4
SI = 128           # s tile (psum partition dim)
NSI = T // SI      # 16
EC = E // 128      # 8 e-chunks
NP = HPC // 2      # 4 head pairs

# ---------------------------------------------------------------------------
# Workarounds for this walrus build: at most ONE sync wait per instruction.
# ---------------------------------------------------------------------------
_PATCHED = False


def _patched_drain_and_barrier(self, tick_clock, wait_clock):
    drain_inst = self.nc.sync.drain(fusable=False)
    wait_clock.add_sem_waits(
        drain_inst.ins, ScopedClock({None: tick_clock.global_clock})
    )
    si = drain_inst.ins.sync_info
    if si is not None and len(si.on_wait) > 1:
        waits = list(si.on_wait)
        drain_inst.ins.sync_info = mybir.SyncInfo(
            on_wait=waits[:1], on_update=list(si.on_update)
        )
        for ofs in range(1, len(waits)):
            extra = self.nc.sync.drain(fusable=False)
            extra.ins.sync_info = mybir.SyncInfo(
                on_wait=waits[ofs : ofs + 1], on_update=[]
            )
    self.nc.all_engine_barrier()
    assert self.sems is not None
    popped = self.nc._tile_sem_poison_stack.pop()
    assert popped is self._sem_poison
    self.nc.clear_and_free_semaphores(list(self.sems.allocated().values()))
    self.nc.all_engine_barrier()


def _install_patches():
    global _PATCHED
    if _PATCHED:
        return
    tile.TileContext._drain_and_barrier = _patched_drain_and_barrier
    _PATCHED = True


def _make_carrier(nc, engine, wait):
    """Wait-only EventSemaphore on `engine` (cheap: ~70ns, no pipe flush)."""
    ev = mybir.InstEventSemaphore(name=f"W-{nc.next_id()}", ins=[], outs=[])
    ev.engine = engine
    ev.sync_info = mybir.SyncInfo(on_wait=[wait], on_update=[])
    return ev


_ENGINE_SEM = {
    "EngineType.PE": "PE",
    "EngineType.DVE": "DVE",
    "EngineType.Activation": "Activation",
    "EngineType.SP": "SP",
    "EngineType.Pool": "Pool",
}
# engines with in-order issue AND in-order completion for these inst types:
# a wait on the engine's own completion sem is redundant. Ldweights excluded
# (the PE reorder window pulls it ahead of in-flight matmuls).
_DROPPABLE = (
    "InstMatmult", "InstActivation", "InstTensorTensor", "InstTensorCopy",
    "InstTensorReduce", "InstMemset", "InstReciprocal", "InstDMACopy",
    "InstCopyPredicated", "InstTensorScalarPtr", "InstTensorScalar",
    "InstCast", "InstDveOp", "InstCustomDve",
)


def _split_multi_waits(nc):
    for bbw in list(nc.bb_map.values()):
        bb = bbw.bb
        insts = bb.instructions
        if not any(
            getattr(i, "sync_info", None) is not None and len(i.sync_info.on_wait) > 1
            for i in insts
        ):
            continue
        out = []
        for inst in insts:
            si = getattr(inst, "sync_info", None)
            waits = list(si.on_wait) if si is not None else []
            if len(waits) > 1:
                own = _ENGINE_SEM.get(str(inst.engine))
                tn = type(inst).__name__
                if own is not None and tn.startswith(_DROPPABLE):
                    waits = [
                        w for w in waits
                        if w.ant_name.rsplit("_", 1)[0] != own
                    ] or waits[-1:]
            if len(waits) > 1:
                for w in waits[:-1]:
                    out.append(_make_carrier(nc, inst.engine, w))
                waits = waits[-1:]
            if si is not None and list(si.on_wait) != waits:
                inst.sync_info = mybir.SyncInfo(
                    on_wait=waits, on_update=list(si.on_update)
                )
            out.append(inst)
        insts[:] = out


# ---------------------------------------------------------------------------
# Mask analysis (host side, 128x128 blocks).
# ---------------------------------------------------------------------------
def _classify_mask(mask):
    """mask: [T, T] bool, mask[t, s]=True means masked (score -> -inf).

    Returns (btab, patterns): btab[i][jj] in {'skip', 'dense', int u};
    patterns[u] is a [128,128] bf16 multiplier in [s, t] orientation."""
    nb = T // 128
    m = np.asarray(mask, dtype=bool)
    patterns = []
    index = {}
    btab = [[None] * nb for _ in range(nb)]
    for i in range(nb):          # s block
        for jj in range(nb):     # t block
            sub = m[jj * 128 : (jj + 1) * 128, i * 128 : (i + 1) * 128]  # [t, s]
            if sub.all():
                btab[i][jj] = "skip"
            elif not sub.any():
                btab[i][jj] = "dense"
            else:
                pat = (~sub).T.astype(NPBF16)  # [s, t] multiplier
                key = pat.tobytes()
                if key not in index:
                    index[key] = len(patterns)
                    patterns.append(pat)
                btab[i][jj] = index[key]
    if not patterns:
        patterns.append(np.ones((128, 128), NPBF16))
    return btab, np.stack(patterns)


# ---------------------------------------------------------------------------
# Kernel builder (SPMD program, identical on all 8 cores).
# ---------------------------------------------------------------------------
def _build(btab, n_pat):
    nc = bass.Bass()
    qT = nc.declare_dram_parameter("qT", [E, T], BF16, isOutput=False)
    kT = nc.declare_dram_parameter("kT", [E, T], BF16, isOutput=False)
    vT = nc.declare_dram_parameter("vT", [E, T], BF16, isOutput=False)
    wq = nc.declare_dram_parameter("wq", [E, DC], BF16, isOutput=False)
    wk = nc.declare_dram_parameter("wk", [E, DC], BF16, isOutput=False)
    wv = nc.declare_dram_parameter("wv", [E, DC], BF16, isOutput=False)
    wpT = nc.declare_dram_parameter("wpT", [DC, E], BF16, isOutput=False)
    pat = nc.declare_dram_parameter("pat", [n_pat * 128, 128], BF16, isOutput=False)
    selbc = nc.declare_dram_parameter("selbc", [HPC, DC], BF16, isOutput=False)
    yT = nc.declare_dram_parameter("yT", [E, T], F32, isOutput=True)
    LN = mybir.ActivationFunctionType.Ln
    import os
    _dbg = os.environ.get("KDBG") == "1"
    if _dbg:
        dbg_u = nc.declare_dram_parameter("dbg_u", [128, 2 * TJ], BF16, isOutput=True)
        dbg_xq = nc.declare_dram_parameter("dbg_xq", [128, T], BF16, isOutput=True)

    EXP = mybir.ActivationFunctionType.Exp

    with ExitStack() as ctx:
        tc = ctx.enter_context(tile.TileContext(nc))
        # SBUF pools
        consts = ctx.enter_context(tc.tile_pool(name="consts", bufs=1))
        streams = ctx.enter_context(tc.tile_pool(name="streams", bufs=1))
        acts = ctx.enter_context(tc.tile_pool(name="acts", bufs=1))
        work = ctx.enter_context(tc.tile_pool(name="work", bufs=1))
        # PSUM pools
        psA = ctx.enter_context(tc.tile_pool(name="psA", bufs=1, space="PSUM"))
        psB = ctx.enter_context(tc.tile_pool(name="psB", bufs=1, space="PSUM"))

        # ---- constants ----
        wq_sb = [consts.tile([128, DC], BF16, tag=f"wq{e}", name=f"wq{e}", bufs=1) for e in range(EC)]
        wk_sb = [consts.tile([128, DC], BF16, tag=f"wk{e}", name=f"wk{e}", bufs=1) for e in range(EC)]
        wv_sb = [consts.tile([128, DC], BF16, tag=f"wv{e}", name=f"wv{e}", bufs=1) for e in range(EC)]
        wp_sb = [consts.tile([128, E], BF16, tag=f"wp{p}", name=f"wp{p}", bufs=1) for p in range(NP)]
        pat_sb = [consts.tile([128, 128], BF16, tag=f"pat{u}", name=f"pat{u}", bufs=1) for u in range(n_pat)]
        selbc_sb = consts.tile([HPC, DC], BF16, tag="selbc", name="selbc", bufs=1)

        # ---- persistent activations ----
        xq_sb = [acts.tile([128, T], BF16, tag=f"xq{p}", name=f"xq{p}", bufs=1) for p in range(NP)]
        xk_sb = [acts.tile([128, T], BF16, tag=f"xk{p}", name=f"xk{p}", bufs=1) for p in range(NP)]
        # xv tiles: per s-tile, heads laid out as 8 x (64 cols xv | 1 col ones)
        xv_sb = [acts.tile([128, HPC * 65], BF16, tag=f"xv{i}", name=f"xv{i}", bufs=1) for i in range(NSI)]
        for i in range(NSI):
            nc.vector.memset(
                xv_sb[i][:].rearrange("p (h x) -> p h x", x=65)[:, :, 64:65], 1.0
            )
        osc_sb_all = [
            [acts.tile([128, TJ], BF16, tag=f"osc{p}_{jj}", name=f"osc{p}_{jj}", bufs=1)
             for p in range(NP)]
            for jj in range(2)
        ]

        # ---- stream tiles (allocated at DMA-emission time) ----
        qs_tiles = {}
        ks_tiles = {}
        vs_tiles = {}

        def dma_q(j):
            jt = slice(j * TJ, (j + 1) * TJ)
            qs = [streams.tile([128, TJ], BF16, tag=f"qs{e}", name=f"qs{e}_{j}", bufs=3) for e in range(EC)]
            for e in range(EC):
                nc.sync.dma_start(out=qs[e][:], in_=qT[e * 128 : (e + 1) * 128, jt])
            qs_tiles[j] = qs

        def dma_k(j):
            # all four k tiles stay live (every head-pair chunk consumes them)
            jt = slice(j * TJ, (j + 1) * TJ)
            ks = [streams.tile([128, TJ], BF16, tag=f"ks{e}", name=f"ks{e}_{j}", bufs=4) for e in range(EC)]
            for e in range(EC):
                nc.sync.dma_start(out=ks[e][:], in_=kT[e * 128 : (e + 1) * 128, jt])
            ks_tiles[j] = ks

        def dma_vs(jp):
            jt = slice(jp * TJ, (jp + 1) * TJ)
            vs = [streams.tile([128, TJ], BF16, tag=f"vs{e}", name=f"vs{e}_{jp}", bufs=2) for e in range(EC)]
            for e in range(EC):
                nc.sync.dma_start(out=vs[e][:], in_=vT[e * 128 : (e + 1) * 128, jt])
            vs_tiles[jp] = vs

        # ---- filler emitters (each emits ~one PSUM group of PE work) ----
        def emit_projqk(j, p, which):
            jt = slice(j * TJ, (j + 1) * TJ)
            pc = slice(p * 128, (p + 1) * 128)
            src = qs_tiles[j] if which == "q" else ks_tiles[j]
            wgt = wq_sb if which == "q" else wk_sb
            dst = xq_sb[p] if which == "q" else xk_sb[p]
            ps = psA.tile([128, TJ], F32, tag="mm512", bufs=2)
            for e in range(EC):
                nc.tensor.matmul(
                    ps[:], wgt[e][:, pc], src[e][:],
                    start=(e == 0), stop=(e == EC - 1),
                )
            nc.vector.tensor_copy(dst[:, jt], ps[:])

        def emit_xv(si):
            jp, loc = divmod(si, 4)
            if jp not in vs_tiles:
                dma_vs(jp)
            if loc == 2 and jp + 1 < NJ and jp + 1 not in vs_tiles:
                dma_vs(jp + 1)  # prefetch next tile's v stream
            vs = vs_tiles[jp]
            ps = psA.tile([128, DC], F32, tag="mm512", bufs=2)
            for e in range(EC):
                nc.tensor.matmul(
                    ps[:], vs[e][:, loc * 128 : (loc + 1) * 128], wv_sb[e][:],
                    start=(e == 0), stop=(e == EC - 1),
                )
            nc.vector.tensor_copy(
                xv_sb[si][:].rearrange("p (h x) -> p h x", x=65)[:, :, 0:64],
                ps[:].rearrange("p (h d) -> p h d", h=HPC),
            )

        def emit_y(j, m, osc_tiles):
            jt = slice(j * TJ, (j + 1) * TJ)
            y_ps = psA.tile([128, TJ], F32, tag="mm512", bufs=2, name=f"y_{m}_{j}")
            for p in range(NP):
                nc.tensor.matmul(
                    y_ps[:], wp_sb[p][:, m * 128 : (m + 1) * 128],
                    osc_tiles[p][:],
                    start=(p == 0), stop=(p == NP - 1),
                )
            y_sb = work.tile([128, TJ], F32, tag="y", bufs=2, name=f"ysb_{m}_{j}")
            nc.vector.tensor_copy(y_sb[:], y_ps[:])
            nc.sync.dma_start(out=yT[m * 128 : (m + 1) * 128, jt], in_=y_sb[:])

        # ---- filler queue machinery ----
        # item: (key, cost_ns, fn);  key = (kind, j, p_or_m_or_si)
        queue = deque()
        state = {"deficit": 0.0}

        PROJ_COST = 1730.0
        Y_COST = 870.0

        def queue_proj(j):
            dma_q(j)
            for p in range(NP):
                queue.append((("projq", j, p), PROJ_COST,
                              (lambda jj=j, pp=p: emit_projqk(jj, pp, "q"))))

        def queue_y(j):
            osc_tiles = osc_sb_all[j % 2]
            for m in range(EC):
                queue.append((("y", j, m), Y_COST,
                              (lambda jj=j, mm=m, ot=osc_tiles: emit_y(jj, mm, ot))))

        # -------------------------------------------------------------------
        # Startup DMA: q/k weights + first tile's q/k, then v weights; the
        # vs / pattern / wp / selector loads ride behind.
        # -------------------------------------------------------------------
        jlist = [0, 1, 2, 3]
        j0 = jlist[0]
        # per-chunk interleave: the first projection matmuls can start as soon
        # as (wq[0], qs[0]) land rather than after the full wq block
        qs0 = [streams.tile([128, TJ], BF16, tag=f"qs{e}", name=f"qs{e}_0", bufs=3) for e in range(EC)]
        ks0 = [streams.tile([128, TJ], BF16, tag=f"ks{e}", name=f"ks{e}_0", bufs=4) for e in range(EC)]
        vs0 = [streams.tile([128, TJ], BF16, tag=f"vs{e}", name=f"vs{e}_0", bufs=2) for e in range(EC)]
        for e in range(EC):
            er = slice(e * 128, (e + 1) * 128)
            nc.sync.dma_start(out=wq_sb[e][:], in_=wq[er, :])
            nc.sync.dma_start(out=qs0[e][:], in_=qT[er, 0:TJ])
        for e in range(EC):
            er = slice(e * 128, (e + 1) * 128)
            nc.sync.dma_start(out=wk_sb[e][:], in_=wk[er, :])
            nc.sync.dma_start(out=ks0[e][:], in_=kT[er, 0:TJ])
        for e in range(EC):
            er = slice(e * 128, (e + 1) * 128)
            nc.sync.dma_start(out=wv_sb[e][:], in_=wv[er, :])
            nc.sync.dma_start(out=vs0[e][:], in_=vT[er, 0:TJ])
        qs_tiles[0] = qs0
        ks_tiles[0] = ks0
        vs_tiles[0] = vs0
        for u in range(n_pat):
            nc.sync.dma_start(out=pat_sb[u][:], in_=pat[u * 128 : (u + 1) * 128, :])
        for p in range(NP):
            nc.sync.dma_start(out=wp_sb[p][:], in_=wpT[p * 128 : (p + 1) * 128, :])
        nc.sync.dma_start(out=selbc_sb[:], in_=selbc[:])

        # first tile's own q projection: p0 inline, p1..3 queued (forced at
        # their attention start). ALL k projections are queued (forced per
        # ival by s-block: attention at s-block i needs xk tile i//4). All xv
        # groups queued (forced as needed).
        emit_projqk(j0, 0, "q")
        for p in range(1, NP):
            queue.append((("projq", j0, p), PROJ_COST,
                          (lambda pp=p: emit_projqk(j0, pp, "q"))))
        for p in range(NP):
            for jp in range(NJ):
                queue.append((("projk", jp, p), PROJ_COST,
                              (lambda jj=jp, pp=p: emit_projqk(jj, pp, "k"))))
        xv_done = {"n": 0}

        def force_xv(upto):
            while xv_done["n"] <= upto:
                si = xv_done["n"]
                emit_xv(si)
                state["deficit"] -= PROJ_COST
                xv_done["n"] += 1

        for si in range(NSI):
            queue.append((("xv", si), PROJ_COST, (lambda s=si: emit_xv(s))))

        # pop/force must skip xv items already force-emitted
        def run_item2(item):
            key = item[0]
            if key[0] == "xv":
                si = key[1]
                if si < xv_done["n"]:
                    return  # already force-emitted
                # emit in-order guarantee: force everything up to si
                force_xv(si)
                return
            item[2]()
            state["deficit"] -= item[1]

        def pop_some2():
            while queue and state["deficit"] > 0.0:
                key = queue[0][0]
                if key[0] == "projk" and key[1] not in ks_tiles:
                    break  # its k stream DMA hasn't been issued yet
                run_item2(queue.popleft())

        def force2(pred):
            keep = deque()
            while queue:
                item = queue.popleft()
                if pred(item[0]):
                    run_item2(item)
                else:
                    keep.append(item)
            queue.extend(keep)

        # -------------------------------------------------------------------
        # Main loop: attention per t-tile (descending), filler interleaved.
        # -------------------------------------------------------------------
        for idx, j in enumerate(jlist):
            jt = slice(j * TJ, (j + 1) * TJ)
            if idx + 1 < len(jlist):
                queue_proj(jlist[idx + 1])
                dma_k(jlist[idx + 1])  # prefetch next tile's k stream
            # per (i): local block types for jj = 4j..4j+3
            ivals = []
            for i in range(NSI):
                types = [btab[i][4 * j + bl] for bl in range(4)]
                if all(t == "skip" for t in types):
                    continue
                ivals.append((i, types))
            n_i = len(ivals)

            osc_sb = osc_sb_all[j % 2]
            osb_sb = [
                work.tile([128, TJ], BF16, tag=f"osb{p}", bufs=2, name=f"osb{p}_{j}")
                for p in range(NP)
            ]
            rcat_sb = work.tile([HPC, TJ], F32, tag="rcat", bufs=2, name=f"rcat_{j}")
            for p in range(NP):
                force2(lambda k, jj=j, pp=p: k[0] == "projq" and k[1] == jj and k[2] == pp)
                o_ps = [
                    psB.tile([65, TJ], F32, tag=f"ops{hh}", name=f"ops{hh}_{p}_{j}", bufs=1)
                    for hh in range(2)
                ]
                pend_av = None  # (ii, i, c0, u)
                for ii, (i, types) in enumerate(ivals):
                    # attention at s-block i reads xk tile i//4 for this p
                    force2(lambda k, pp=p, jj=i // 4: k[0] == "projk"
                           and k[2] == pp and k[1] <= jj)
                    # pre-force the next chunk's lead-in projections so its
                    # scores issue immediately at the boundary (no ACT bubble)
                    if ii == n_i - 2:
                        if p + 1 < NP:
                            force2(lambda k, jj=j, pp=p + 1:
                                   (k[0] == "projq" and k[1] == jj and k[2] == pp)
                                   or (k[0] == "projk" and k[1] == 0 and k[2] == pp))
                        elif idx + 1 < len(jlist):
                            force2(lambda k, jj=jlist[idx + 1]:
                                   k[0] == "projq" and k[1] == jj and k[2] == 0)
                    c0 = next(bl for bl in range(4) if types[bl] != "skip")
                    cols = TJ - c0 * 128
                    state["deficit"] += (2 * cols + 352) / 1.2 - 1.25 * cols - 80.0
                    # both heads' S^T side by side in one 2-bank psum tile;
                    # one batched exp over a 3D AP covering both halves
                    st = psA.tile([128, 2 * TJ], F32, tag="st", bufs=2)
                    for hh in range(2):
                        hr = slice(hh * 64, (hh + 1) * 64)
                        nc.tensor.matmul(
                            st[:, hh * TJ + c0 * 128 : (hh + 1) * TJ],
                            xk_sb[p][hr, i * 128 : (i + 1) * 128],
                            xq_sb[p][hr, jt][:, c0 * 128 : TJ],
                            start=True, stop=True,
                        )
                    u = work.tile([128, 2 * TJ], BF16, tag="u", bufs=4)
                    nc.scalar.activation(
                        u[:].rearrange("p (g c) -> p g c", g=2)[:, :, c0 * 128 : TJ],
                        st[:].rearrange("p (g c) -> p g c", g=2)[:, :, c0 * 128 : TJ],
                        EXP, scale=1.0 / 32.0,
                    )
                    if _dbg and j == 3 and p == 0 and ii == 0:
                        nc.sync.dma_start(out=dbg_u[:], in_=u[:])
                    # mask fixup: only the leading block of a run can be mixed;
                    # multiply the pattern in place so AV is one long matmul
                    if isinstance(types[c0], int):
                        for hh in range(2):
                            uo = hh * TJ
                            blk = slice(uo + c0 * 128, uo + (c0 + 1) * 128)
                            nc.vector.tensor_mul(
                                u[:, blk], u[:, blk], pat_sb[types[c0]][:]
                            )
                    assert all(not isinstance(types[bl], int) for bl in range(c0 + 1, 4)), \
                        "mask: mixed block beyond run head"
                    # filler between the score pair and the lagged AV: the PE
                    # chews dense matmuls here while ACT runs exp
                    pop_some2()
                    if p == 0:
                        force_xv(min(i + 1, 4 * j + 3))
                    # lag-1 AV: gives exp(ii) a full ival of slack
                    if pend_av is not None:
                        pii, pi, pc0, pu = pend_av
                        for hh in range(2):
                            h = 2 * p + hh
                            nc.tensor.matmul(
                                o_ps[hh][:, pc0 * 128 : TJ],
                                xv_sb[pi][:, h * 65 : h * 65 + 65],
                                pu[:, hh * TJ + pc0 * 128 : hh * TJ + TJ],
                                start=(pii == 0), stop=False,
                                skip_group_check=True,
                            )
                    pend_av = (ii, i, c0, u)
                # final AV (stop=True)
                pii, pi, pc0, pu = pend_av
                for hh in range(2):
                    h = 2 * p + hh
                    nc.tensor.matmul(
                        o_ps[hh][:, pc0 * 128 : TJ],
                        xv_sb[pi][:, h * 65 : h * 65 + 65],
                        pu[:, hh * TJ + pc0 * 128 : hh * TJ + TJ],
                        start=(pii == 0), stop=True,
                        skip_group_check=True,
                    )
                # ---- stage rowsums + o rows so o_ps can release ----
                for hh in range(2):
                    h = 2 * p + hh
                    rsb = work.tile([1, TJ], F32, tag="rsb", bufs=4)
                    nc.vector.tensor_copy(rsb[:], o_ps[hh][64:65, :])
                    nc.sync.dma_start(out=rcat_sb[h : h + 1, :], in_=rsb[:])
                    nc.vector.tensor_copy(
                        osb_sb[p][hh * 64 : (hh + 1) * 64, :], o_ps[hh][0:64, :]
                    )
            # ---- tile tail: 1/r = exp(-ln r) on ACT, emitted inline (ACT
            # reaches it right after this tile's last exp, when the rowsum
            # DMAs have landed). The per-p broadcast matmul + osc products
            # are deferred into the next tile via the filler queue.
            lnr = work.tile([HPC, TJ], F32, tag="lnr", bufs=2, name=f"lnr_{j}")
            nc.scalar.activation(lnr[:], rcat_sb[:], LN)
            rr = work.tile([HPC, TJ], BF16, tag="rr", bufs=2, name=f"rr_{j}")
            nc.scalar.activation(rr[:], lnr[:], EXP, scale=-1.0)

            def emit_tail(jj, rrt, osbs, oscs):
                for p in range(NP):
                    rb_ps = psA.tile([128, TJ], F32, tag="mm512", bufs=2,
                                     name=f"rb_{p}_{jj}")
                    nc.tensor.matmul(
                        rb_ps[:], selbc_sb[:, p * 128 : (p + 1) * 128], rrt[:],
                        start=True, stop=True,
                    )
                    nc.vector.tensor_mul(oscs[p][:], osbs[p][:], rb_ps[:])

            queue.append((("y", j, -1), 1700.0,
                          (lambda jj=j, rrt=rr, ob=osb_sb, oc=osc_sb:
                           emit_tail(jj, rrt, ob, oc))))
            # output projection for this tile rides the filler queue
            queue_y(j)
            # keep osc ping-pong safe: before tile `nxt` rewrites osc[nxt%2],
            # any queued y reading that buffer must drain
            if idx + 1 < len(jlist):
                nxt = jlist[idx + 1]
                force2(lambda k, par=(nxt % 2): k[0] == "y" and (k[1] % 2) == par)

        # drain
        while queue:
            run_item2(queue.popleft())

    _split_multi_waits(nc)
    return nc


_SELBC = np.zeros((HPC, DC), NPBF16)
for _p in range(HPC // 2):
    _SELBC[2 * _p, _p * 128 : _p * 128 + 64] = 1.0
    _SELBC[2 * _p + 1, _p * 128 + 64 : _p * 128 + 128] = 1.0

_CACHE = {}


def _get_program(mask):
    key = np.asarray(mask, dtype=bool).tobytes()
    prog = _CACHE.get(key)
    if prog is None:
        _install_patches()
        btab, patterns = _classify_mask(mask)
        nc = _build(btab, len(patterns))
        prog = (nc, patterns)
        _CACHE[key] = prog
    return prog


def _prepare(k, q, v, mask, Wk, Wq, Wv, Wp):
    """Build (cached) the SPMD program and the 8 per-core input maps."""
    k = np.asarray(k, np.float32)
    q = np.asarray(q, np.float32)
    v = np.asarray(v, np.float32)
    Wk = np.asarray(Wk, np.float32)
    Wq = np.asarray(Wq, np.float32)
    Wv = np.asarray(Wv, np.float32)
    Wp = np.asarray(Wp, np.float32)

    nc, patterns = _get_program(mask)
    patflat = np.ascontiguousarray(patterns.reshape(-1, 128))

    def tr(x):  # [T, E] f32 -> [E, T] bf16 contiguous
        return np.ascontiguousarray(x.astype(NPBF16).T)

    def wcat(W, half):  # [H, E, D] -> [E, 512] bf16 for this half's 8 heads
        return np.ascontiguousarray(
            W[half * HPC : (half + 1) * HPC].transpose(1, 0, 2).reshape(E, DC)
        ).astype(NPBF16)

    in_maps = []
    for c in range(8):
        b, half = divmod(c, 2)
        off = half * DC
        in_maps.append(
            {
                "qT": tr(q[b]),
                "kT": tr(k[b]),
                "vT": tr(v[b]),
                "wq": wcat(Wq, half),
                "wk": wcat(Wk, half),
                "wv": wcat(Wv, half),
                "wpT": np.ascontiguousarray(Wp[:, off : off + DC].T).astype(NPBF16),
                "pat": patflat,
                "selbc": _SELBC,
            }
        )
    return nc, in_maps


def kernel(k, q, v, mask, Wk, Wq, Wv, Wp, bp):
    bp = np.asarray(bp, np.float32)
    nc, in_maps = _prepare(k, q, v, mask, Wk, Wq, Wv, Wp)
    res = run_bass_kernel_spmd(nc, in_maps, list(range(8)))
    out = np.empty((B, T, E), np.float32)
    for b in range(B):
        yt = res.results[2 * b]["yT"] + res.results[2 * b + 1]["yT"]
        out[b] = yt.T + bp[None, :]
    return out


# revision 46
# speedup vs baseline: 1.1057x; 1.1057x over previous
"""Multi-head causal attention (B=4, T=2048, E=1024, H=16, D=64) on 8 trn2
NeuronCores via Bass/Tile.

Sharding: core c handles batch b = c//2 and heads [half*8, half*8+8), half =
c%2. Each core computes its 8 heads' attention and a partial output
projection Y^T = Wp_slice^T-contraction over its heads; the host sums the two
half partials per batch, transposes, and adds the bias.

On-device layout is "transposed": activations are [feature, token] so every
matmul contracts over the partition dim. Softmax denominators come from a
ones-column appended to the stationary V operand (M=65 matmuls); masking is
applied block-wise (128x128) in place on the exp'd scores so each AV tile is
a single long matmul. The t-tiles are processed in descending order with a
filler queue (projections / output-projection groups) interleaved between
attention ivals: the attention inner loop is ACT(exp)-bound, so the PE gets
dense matmul filler while exp runs. No max-subtraction is needed: scores are
~N(0, 0.083^2).
"""
import numpy as np
import ml_dtypes
from collections import deque
from contextlib import ExitStack

import concourse.bass as bass
import concourse.mybir as mybir
import concourse.tile as tile
from concourse.bass_utils import run_bass_kernel_spmd
from concourse.vector_clock import ScopedClock

BF16 = mybir.dt.bfloat16
F32 = mybir.dt.float32
F8 = mybir.dt.float8e4
NPBF16 = ml_dtypes.bfloat16
NPF8 = ml_dtypes.float8_e4m3fn
DBLROW = mybir.MatmulPerfMode.DoubleRow

B, T, E, H, D = 4, 2048, 1024, 16, 64
HPC = 8            # heads per core
DC = HPC * D       # 512: stacked head dim per core
TJ = 512           # t tile (matmul free dim)
NJ = T // TJ       # 4
SI = 128           # s tile (psum partition dim)
NSI = T // SI      # 16
EC = E // 128      # 8 e-chunks
NP = HPC // 2      # 4 head pairs

# ---------------------------------------------------------------------------
# Workarounds for this walrus build: at most ONE sync wait per instruction.
# ---------------------------------------------------------------------------
_PATCHED = False


def _patched_drain_and_barrier(self, tick_clock, wait_clock):
    drain_inst = self.nc.sync.drain(fusable=False)
    wait_clock.add_sem_waits(
        drain_inst.ins, ScopedClock({None: tick_clock.global_clock})
    )
    si = drain_inst.ins.sync_info
    if si is not None and len(si.on_wait) > 1:
        waits = list(si.on_wait)
        drain_inst.ins.sync_info = mybir.SyncInfo(
            on_wait=waits[:1], on_update=list(si.on_update)
        )
        for ofs in range(1, len(waits)):
            extra = self.nc.sync.drain(fusable=False)
            extra.ins.sync_info = mybir.SyncInfo(
                on_wait=waits[ofs : ofs + 1], on_update=[]
            )
    self.nc.all_engine_barrier()
    assert self.sems is not None
    popped = self.nc._tile_sem_poison_stack.pop()
    assert popped is self._sem_poison
    self.nc.clear_and_free_semaphores(list(self.sems.allocated().values()))
    self.nc.all_engine_barrier()


def _install_patches():
    global _PATCHED
    if _PATCHED:
        return
    tile.TileContext._drain_and_barrier = _patched_drain_and_barrier
    _PATCHED = True


def _make_carrier(nc, engine, wait):
    """Wait-only EventSemaphore on `engine` (cheap: ~70ns, no pipe flush)."""
    ev = mybir.InstEventSemaphore(name=f"W-{nc.next_id()}", ins=[], outs=[])
    ev.engine = engine
    ev.sync_info = mybir.SyncInfo(on_wait=[wait], on_update=[])
    return ev


_ENGINE_SEM = {
    "EngineType.PE": "PE",
    "EngineType.DVE": "DVE",
    "EngineType.Activation": "Activation",
    "EngineType.SP": "SP",
    "EngineType.Pool": "Pool",
}
# engines with in-order issue AND in-order completion for these inst types:
# a wait on the engine's own completion sem is redundant. Ldweights excluded
# (the PE reorder window pulls it ahead of in-flight matmuls).
_DROPPABLE = (
    "InstMatmult", "InstActivation", "InstTensorTensor", "InstTensorCopy",
    "InstTensorReduce", "InstMemset", "InstReciprocal", "InstDMACopy",
    "InstCopyPredicated", "InstTensorScalarPtr", "InstTensorScalar",
    "InstCast", "InstDveOp", "InstCustomDve",
)


def _split_multi_waits(nc):
    for bbw in list(nc.bb_map.values()):
        bb = bbw.bb
        insts = bb.instructions
        if not any(
            getattr(i, "sync_info", None) is not None and len(i.sync_info.on_wait) > 1
            for i in insts
        ):
            continue
        out = []
        for inst in insts:
            si = getattr(inst, "sync_info", None)
            waits = list(si.on_wait) if si is not None else []
            if len(waits) > 1:
                own = _ENGINE_SEM.get(str(inst.engine))
                tn = type(inst).__name__
                if own is not None and tn.startswith(_DROPPABLE):
                    waits = [
                        w for w in waits
                        if w.ant_name.rsplit("_", 1)[0] != own
                    ] or waits[-1:]
            if len(waits) > 1:
                for w in waits[:-1]:
                    out.append(_make_carrier(nc, inst.engine, w))
                waits = waits[-1:]
            if si is not None and list(si.on_wait) != waits:
                inst.sync_info = mybir.SyncInfo(
                    on_wait=waits, on_update=list(si.on_update)
                )
            out.append(inst)
        insts[:] = out


# ---------------------------------------------------------------------------
# Mask analysis (host side, 128x128 blocks).
# ---------------------------------------------------------------------------
def _classify_mask(mask):
    """mask: [T, T] bool, mask[t, s]=True means masked (score -> -inf).

    Returns (btab, patterns): btab[i][jj] in {'skip', 'dense', int u};
    patterns[u] is a [128,128] bf16 multiplier in [s, t] orientation."""
    nb = T // 128
    m = np.asarray(mask, dtype=bool)
    patterns = []
    index = {}
    btab = [[None] * nb for _ in range(nb)]
    for i in range(nb):          # s block
        for jj in range(nb):     # t block
            sub = m[jj * 128 : (jj + 1) * 128, i * 128 : (i + 1) * 128]  # [t, s]
            if sub.all():
                btab[i][jj] = "skip"
            elif not sub.any():
                btab[i][jj] = "dense"
            else:
                pat = (~sub).T.astype(NPBF16)  # [s, t] multiplier
                key = pat.tobytes()
                if key not in index:
                    index[key] = len(patterns)
                    patterns.append(pat)
                btab[i][jj] = index[key]
    if not patterns:
        patterns.append(np.ones((128, 128), NPBF16))
    return btab, np.stack(patterns)


# ---------------------------------------------------------------------------
# Kernel builder (SPMD program, identical on all 8 cores).
# ---------------------------------------------------------------------------
def _build(btab, n_pat):
    nc = bass.Bass()
    qT = nc.declare_dram_parameter("qT", [E, T], F8, isOutput=False)
    kT = nc.declare_dram_parameter("kT", [E, T], F8, isOutput=False)
    vT = nc.declare_dram_parameter("vT", [E, T], BF16, isOutput=False)
    wq = nc.declare_dram_parameter("wq", [E, DC], F8, isOutput=False)
    wk = nc.declare_dram_parameter("wk", [E, DC], F8, isOutput=False)
    wv = nc.declare_dram_parameter("wv", [E, DC], BF16, isOutput=False)
    wpT = nc.declare_dram_parameter("wpT", [DC, E], BF16, isOutput=False)
    pat = nc.declare_dram_parameter("pat", [n_pat * 128, 128], BF16, isOutput=False)
    selbc = nc.declare_dram_parameter("selbc", [HPC, DC], BF16, isOutput=False)
    yT = nc.declare_dram_parameter("yT", [E, T], F32, isOutput=True)
    LN = mybir.ActivationFunctionType.Ln
    import os
    _dbg = os.environ.get("KDBG") == "1"
    if _dbg:
        dbg_u = nc.declare_dram_parameter("dbg_u", [128, 2 * TJ], BF16, isOutput=True)
        dbg_xq = nc.declare_dram_parameter("dbg_xq", [128, T], BF16, isOutput=True)

    EXP = mybir.ActivationFunctionType.Exp

    with ExitStack() as ctx:
        tc = ctx.enter_context(tile.TileContext(nc))
        # SBUF pools
        consts = ctx.enter_context(tc.tile_pool(name="consts", bufs=1))
        streams = ctx.enter_context(tc.tile_pool(name="streams", bufs=1))
        acts = ctx.enter_context(tc.tile_pool(name="acts", bufs=1))
        work = ctx.enter_context(tc.tile_pool(name="work", bufs=1))
        # PSUM pools
        psA = ctx.enter_context(tc.tile_pool(name="psA", bufs=1, space="PSUM"))
        psB = ctx.enter_context(tc.tile_pool(name="psB", bufs=1, space="PSUM"))

        # ---- constants ----
        # q/k projection weights in fp8, paired e-chunks side by side for
        # DoubleRow matmuls (contraction 256 per instruction)
        EP = EC // 2
        wq_sb = [consts.tile([128, 2 * DC], F8, tag=f"wq{ep}", name=f"wq{ep}", bufs=1) for ep in range(EP)]
        wk_sb = [consts.tile([128, 2 * DC], F8, tag=f"wk{ep}", name=f"wk{ep}", bufs=1) for ep in range(EP)]
        wv_sb = [consts.tile([128, DC], BF16, tag=f"wv{e}", name=f"wv{e}", bufs=1) for e in range(EC)]
        wp_sb = [consts.tile([128, E], BF16, tag=f"wp{p}", name=f"wp{p}", bufs=1) for p in range(NP)]
        pat_sb = [consts.tile([128, 128], BF16, tag=f"pat{u}", name=f"pat{u}", bufs=1) for u in range(n_pat)]
        selbc_sb = consts.tile([HPC, DC], BF16, tag="selbc", name="selbc", bufs=1)

        # ---- persistent activations ----
        xq_sb = [acts.tile([128, T], BF16, tag=f"xq{p}", name=f"xq{p}", bufs=1) for p in range(NP)]
        xk_sb = [acts.tile([128, T], BF16, tag=f"xk{p}", name=f"xk{p}", bufs=1) for p in range(NP)]
        # xv tiles: per s-tile, heads laid out as 8 x (64 cols xv | 1 col ones)
        xv_sb = [acts.tile([128, HPC * 65], BF16, tag=f"xv{i}", name=f"xv{i}", bufs=1) for i in range(NSI)]
        for i in range(NSI):
            nc.vector.memset(
                xv_sb[i][:].rearrange("p (h x) -> p h x", x=65)[:, :, 64:65], 1.0
            )
        osc_sb_all = [
            [acts.tile([128, TJ], BF16, tag=f"osc{p}_{jj}", name=f"osc{p}_{jj}", bufs=1)
             for p in range(NP)]
            for jj in range(2)
        ]

        # ---- stream tiles (allocated at DMA-emission time) ----
        qs_tiles = {}
        ks_tiles = {}
        vs_tiles = {}

        def dma_q(j):
            jt = slice(j * TJ, (j + 1) * TJ)
            qs = [streams.tile([128, 2 * TJ], F8, tag=f"qs{ep}", name=f"qs{ep}_{j}", bufs=3) for ep in range(EP)]
            for ep in range(EP):
                nc.sync.dma_start(out=qs[ep][:, 0:TJ], in_=qT[2 * ep * 128 : (2 * ep + 1) * 128, jt])
                nc.sync.dma_start(out=qs[ep][:, TJ : 2 * TJ], in_=qT[(2 * ep + 1) * 128 : (2 * ep + 2) * 128, jt])
            qs_tiles[j] = qs

        def dma_k(j):
            # all four k tiles stay live (every head-pair chunk consumes them)
            jt = slice(j * TJ, (j + 1) * TJ)
            ks = [streams.tile([128, 2 * TJ], F8, tag=f"ks{ep}", name=f"ks{ep}_{j}", bufs=4) for ep in range(EP)]
            for ep in range(EP):
                nc.sync.dma_start(out=ks[ep][:, 0:TJ], in_=kT[2 * ep * 128 : (2 * ep + 1) * 128, jt])
                nc.sync.dma_start(out=ks[ep][:, TJ : 2 * TJ], in_=kT[(2 * ep + 1) * 128 : (2 * ep + 2) * 128, jt])
            ks_tiles[j] = ks

        def dma_vs(jp):
            jt = slice(jp * TJ, (jp + 1) * TJ)
            vs = [streams.tile([128, TJ], BF16, tag=f"vs{e}", name=f"vs{e}_{jp}", bufs=2) for e in range(EC)]
            for e in range(EC):
                nc.sync.dma_start(out=vs[e][:], in_=vT[e * 128 : (e + 1) * 128, jt])
            vs_tiles[jp] = vs

        # ---- filler emitters (each emits ~one PSUM group of PE work) ----
        def emit_projqk(j, p, which):
            jt = slice(j * TJ, (j + 1) * TJ)
            src = qs_tiles[j] if which == "q" else ks_tiles[j]
            wgt = wq_sb if which == "q" else wk_sb
            dst = xq_sb[p] if which == "q" else xk_sb[p]
            ps = psA.tile([128, TJ], F32, tag="mm512", bufs=2)
            for ep in range(EP):
                lhsT = wgt[ep][:].rearrange("x (two m) -> x two m", two=2)[
                    :, :, p * 128 : (p + 1) * 128
                ]
                rhs = src[ep][:].rearrange("x (two n) -> x two n", two=2)
                nc.tensor.matmul(
                    ps[:], lhsT, rhs,
                    start=(ep == 0), stop=(ep == EP - 1),
                    perf_mode=DBLROW,
                )
            nc.vector.tensor_copy(dst[:, jt], ps[:])

        def emit_xv(si):
            jp, loc = divmod(si, 4)
            if jp not in vs_tiles:
                dma_vs(jp)
            if loc == 2 and jp + 1 < NJ and jp + 1 not in vs_tiles:
                dma_vs(jp + 1)  # prefetch next tile's v stream
            vs = vs_tiles[jp]
            ps = psA.tile([128, DC], F32, tag="mm512", bufs=2)
            for e in range(EC):
                nc.tensor.matmul(
                    ps[:], vs[e][:, loc * 128 : (loc + 1) * 128], wv_sb[e][:],
                    start=(e == 0), stop=(e == EC - 1),
                )
            nc.vector.tensor_copy(
                xv_sb[si][:].rearrange("p (h x) -> p h x", x=65)[:, :, 0:64],
                ps[:].rearrange("p (h d) -> p h d", h=HPC),
            )

        def emit_y(j, m, osc_tiles):
            jt = slice(j * TJ, (j + 1) * TJ)
            y_ps = psA.tile([128, TJ], F32, tag="mm512", bufs=2, name=f"y_{m}_{j}")
            for p in range(NP):
                nc.tensor.matmul(
                    y_ps[:], wp_sb[p][:, m * 128 : (m + 1) * 128],
                    osc_tiles[p][:],
                    start=(p == 0), stop=(p == NP - 1),
                )
            y_sb = work.tile([128, TJ], F32, tag="y", bufs=2, name=f"ysb_{m}_{j}")
            nc.vector.tensor_copy(y_sb[:], y_ps[:])
            nc.sync.dma_start(out=yT[m * 128 : (m + 1) * 128, jt], in_=y_sb[:])

        # ---- filler queue machinery ----
        # item: (key, cost_ns, fn);  key = (kind, j, p_or_m_or_si)
        queue = deque()
        state = {"deficit": 0.0}

        PROJ_COST = 1730.0
        Y_COST = 870.0

        def queue_proj(j):
            dma_q(j)
            for p in range(NP):
                queue.append((("projq", j, p), PROJ_COST,
                              (lambda jj=j, pp=p: emit_projqk(jj, pp, "q"))))

        def queue_y(j):
            osc_tiles = osc_sb_all[j % 2]
            for m in range(EC):
                queue.append((("y", j, m), Y_COST,
                              (lambda jj=j, mm=m, ot=osc_tiles: emit_y(jj, mm, ot))))

        # -------------------------------------------------------------------
        # Startup DMA: q/k weights + first tile's q/k, then v weights; the
        # vs / pattern / wp / selector loads ride behind.
        # -------------------------------------------------------------------
        jlist = [0, 1, 2, 3]
        j0 = jlist[0]
        # per-chunk interleave: the first projection matmuls can start as soon
        # as (wq[0], qs[0]) land rather than after the full wq block
        for ep in range(EP):
            for h in range(2):
                er = slice((2 * ep + h) * 128, (2 * ep + h + 1) * 128)
                nc.sync.dma_start(out=wq_sb[ep][:, h * DC : (h + 1) * DC], in_=wq[er, :])
        dma_q(0)
        for ep in range(EP):
            for h in range(2):
                er = slice((2 * ep + h) * 128, (2 * ep + h + 1) * 128)
                nc.sync.dma_start(out=wk_sb[ep][:, h * DC : (h + 1) * DC], in_=wk[er, :])
        dma_k(0)
        for e in range(EC):
            er = slice(e * 128, (e + 1) * 128)
            nc.sync.dma_start(out=wv_sb[e][:], in_=wv[er, :])
        dma_vs(0)
        for u in range(n_pat):
            nc.sync.dma_start(out=pat_sb[u][:], in_=pat[u * 128 : (u + 1) * 128, :])
        for p in range(NP):
            nc.sync.dma_start(out=wp_sb[p][:], in_=wpT[p * 128 : (p + 1) * 128, :])
        nc.sync.dma_start(out=selbc_sb[:], in_=selbc[:])

        # first tile's own q projection: p0 inline, p1..3 queued (forced at
        # their attention start). ALL k projections are queued (forced per
        # ival by s-block: attention at s-block i needs xk tile i//4). All xv
        # groups queued (forced as needed).
        emit_projqk(j0, 0, "q")
        for p in range(1, NP):
            queue.append((("projq", j0, p), PROJ_COST,
                          (lambda pp=p: emit_projqk(j0, pp, "q"))))
        for p in range(NP):
            for jp in range(NJ):
                queue.append((("projk", jp, p), PROJ_COST,
                              (lambda jj=jp, pp=p: emit_projqk(jj, pp, "k"))))
        xv_done = {"n": 0}

        def force_xv(upto):
            while xv_done["n"] <= upto:
                si = xv_done["n"]
                emit_xv(si)
                state["deficit"] -= PROJ_COST
                xv_done["n"] += 1

        for si in range(NSI):
            queue.append((("xv", si), PROJ_COST, (lambda s=si: emit_xv(s))))

        # pop/force must skip xv items already force-emitted
        def run_item2(item):
            key = item[0]
            if key[0] == "xv":
                si = key[1]
                if si < xv_done["n"]:
                    return  # already force-emitted
                # emit in-order guarantee: force everything up to si
                force_xv(si)
                return
            item[2]()
            state["deficit"] -= item[1]

        def pop_some2():
            while queue and state["deficit"] > 0.0:
                key = queue[0][0]
                if key[0] == "projk" and key[1] not in ks_tiles:
                    break  # its k stream DMA hasn't been issued yet
                run_item2(queue.popleft())

        def force2(pred):
            keep = deque()
            while queue:
                item = queue.popleft()
                if pred(item[0]):
                    run_item2(item)
                else:
                    keep.append(item)
            queue.extend(keep)

        # -------------------------------------------------------------------
        # Main loop: attention per t-tile (descending), filler interleaved.
        # -------------------------------------------------------------------
        for idx, j in enumerate(jlist):
            jt = slice(j * TJ, (j + 1) * TJ)
            if idx + 1 < len(jlist):
                queue_proj(jlist[idx + 1])
                dma_k(jlist[idx + 1])  # prefetch next tile's k stream
            # per (i): local block types for jj = 4j..4j+3
            ivals = []
            for i in range(NSI):
                types = [btab[i][4 * j + bl] for bl in range(4)]
                if all(t == "skip" for t in types):
                    continue
                ivals.append((i, types))
            n_i = len(ivals)

            osc_sb = osc_sb_all[j % 2]
            osb_sb = [
                work.tile([128, TJ], BF16, tag=f"osb{p}", bufs=2, name=f"osb{p}_{j}")
                for p in range(NP)
            ]
            rcat_sb = work.tile([HPC, TJ], F32, tag="rcat", bufs=2, name=f"rcat_{j}")
            for p in range(NP):
                force2(lambda k, jj=j, pp=p: k[0] == "projq" and k[1] == jj and k[2] == pp)
                o_ps = [
                    psB.tile([65, TJ], F32, tag=f"ops{hh}", name=f"ops{hh}_{p}_{j}", bufs=1)
                    for hh in range(2)
                ]
                pend_av = None  # (ii, i, c0, u)
                for ii, (i, types) in enumerate(ivals):
                    # attention at s-block i reads xk tile i//4 for this p
                    force2(lambda k, pp=p, jj=i // 4: k[0] == "projk"
                           and k[2] == pp and k[1] <= jj)
                    # pre-force the next chunk's lead-in projections so its
                    # scores issue immediately at the boundary (no ACT bubble)
                    if ii == n_i - 2:
                        if p + 1 < NP:
                            force2(lambda k, jj=j, pp=p + 1:
                                   (k[0] == "projq" and k[1] == jj and k[2] == pp)
                                   or (k[0] == "projk" and k[1] == 0 and k[2] == pp))
                        elif idx + 1 < len(jlist):
                            force2(lambda k, jj=jlist[idx + 1]:
                                   k[0] == "projq" and k[1] == jj and k[2] == 0)
                    c0 = next(bl for bl in range(4) if types[bl] != "skip")
                    cols = TJ - c0 * 128
                    state["deficit"] += (2 * cols + 352) / 1.2 - 1.25 * cols - 80.0
                    # both heads' S^T side by side in one 2-bank psum tile;
                    # one batched exp over a 3D AP covering both halves
                    st = psA.tile([128, 2 * TJ], F32, tag="st", bufs=2)
                    for hh in range(2):
                        hr = slice(hh * 64, (hh + 1) * 64)
                        nc.tensor.matmul(
                            st[:, hh * TJ + c0 * 128 : (hh + 1) * TJ],
                            xk_sb[p][hr, i * 128 : (i + 1) * 128],
                            xq_sb[p][hr, jt][:, c0 * 128 : TJ],
                            start=True, stop=True,
                        )
                    u = work.tile([128, 2 * TJ], BF16, tag="u", bufs=4)
                    nc.scalar.activation(
                        u[:].rearrange("p (g c) -> p g c", g=2)[:, :, c0 * 128 : TJ],
                        st[:].rearrange("p (g c) -> p g c", g=2)[:, :, c0 * 128 : TJ],
                        EXP, scale=1.0 / 32.0,
                    )
                    if _dbg and j == 3 and p == 0 and ii == 0:
                        nc.sync.dma_start(out=dbg_u[:], in_=u[:])
                    # mask fixup: only the leading block of a run can be mixed;
                    # multiply the pattern in place so AV is one long matmul
                    if isinstance(types[c0], int):
                        for hh in range(2):
                            uo = hh * TJ
                            blk = slice(uo + c0 * 128, uo + (c0 + 1) * 128)
                            nc.vector.tensor_mul(
                                u[:, blk], u[:, blk], pat_sb[types[c0]][:]
                            )
                    assert all(not isinstance(types[bl], int) for bl in range(c0 + 1, 4)), \
                        "mask: mixed block beyond run head"
                    # filler between the score pair and the lagged AV: the PE
                    # chews dense matmuls here while ACT runs exp
                    pop_some2()
                    if p == 0:
                        force_xv(min(i + 1, 4 * j + 3))
                    # lag-1 AV: gives exp(ii) a full ival of slack
                    if pend_av is not None:
                        pii, pi, pc0, pu = pend_av
                        for hh in range(2):
                            h = 2 * p + hh
                            nc.tensor.matmul(
                                o_ps[hh][:, pc0 * 128 : TJ],
                                xv_sb[pi][:, h * 65 : h * 65 + 65],
                                pu[:, hh * TJ + pc0 * 128 : hh * TJ + TJ],
                                start=(pii == 0), stop=False,
                                skip_group_check=True,
                            )
                    pend_av = (ii, i, c0, u)
                # final AV (stop=True)
                pii, pi, pc0, pu = pend_av
                for hh in range(2):
                    h = 2 * p + hh
                    nc.tensor.matmul(
                        o_ps[hh][:, pc0 * 128 : TJ],
                        xv_sb[pi][:, h * 65 : h * 65 + 65],
                        pu[:, hh * TJ + pc0 * 128 : hh * TJ + TJ],
                        start=(pii == 0), stop=True,
                        skip_group_check=True,
                    )
                # ---- stage rowsums + o rows so o_ps can release ----
                for hh in range(2):
                    h = 2 * p + hh
                    rsb = work.tile([1, TJ], F32, tag="rsb", bufs=4)
                    nc.vector.tensor_copy(rsb[:], o_ps[hh][64:65, :])
                    nc.sync.dma_start(out=rcat_sb[h : h + 1, :], in_=rsb[:])
                    nc.vector.tensor_copy(
                        osb_sb[p][hh * 64 : (hh + 1) * 64, :], o_ps[hh][0:64, :]
                    )
            # ---- tile tail: 1/r = exp(-ln r) on ACT, emitted inline (ACT
            # reaches it right after this tile's last exp, when the rowsum
            # DMAs have landed). The per-p broadcast matmul + osc products
            # are deferred into the next tile via the filler queue.
            lnr = work.tile([HPC, TJ], F32, tag="lnr", bufs=2, name=f"lnr_{j}")
            nc.scalar.activation(lnr[:], rcat_sb[:], LN)
            rr = work.tile([HPC, TJ], BF16, tag="rr", bufs=2, name=f"rr_{j}")
            nc.scalar.activation(rr[:], lnr[:], EXP, scale=-1.0)

            def emit_tail(jj, rrt, osbs, oscs):
                for p in range(NP):
                    rb_ps = psA.tile([128, TJ], F32, tag="mm512", bufs=2,
                                     name=f"rb_{p}_{jj}")
                    nc.tensor.matmul(
                        rb_ps[:], selbc_sb[:, p * 128 : (p + 1) * 128], rrt[:],
                        start=True, stop=True,
                    )
                    nc.vector.tensor_mul(oscs[p][:], osbs[p][:], rb_ps[:])

            queue.append((("y", j, -1), 1700.0,
                          (lambda jj=j, rrt=rr, ob=osb_sb, oc=osc_sb:
                           emit_tail(jj, rrt, ob, oc))))
            # output projection for this tile rides the filler queue
            queue_y(j)
            # keep osc ping-pong safe: before tile `nxt` rewrites osc[nxt%2],
            # any queued y reading that buffer must drain
            if idx + 1 < len(jlist):
                nxt = jlist[idx + 1]
                force2(lambda k, par=(nxt % 2): k[0] == "y" and (k[1] % 2) == par)

        # drain
        while queue:
            run_item2(queue.popleft())

    _split_multi_waits(nc)
    return nc


_SELBC = np.zeros((HPC, DC), NPBF16)
for _p in range(HPC // 2):
    _SELBC[2 * _p, _p * 128 : _p * 128 + 64] = 1.0
    _SELBC[2 * _p + 1, _p * 128 + 64 : _p * 128 + 128] = 1.0

_CACHE = {}


def _get_program(mask):
    key = np.asarray(mask, dtype=bool).tobytes()
    prog = _CACHE.get(key)
    if prog is None:
        _install_patches()
        btab, patterns = _classify_mask(mask)
        nc = _build(btab, len(patterns))
        prog = (nc, patterns)
        _CACHE[key] = prog
    return prog


def _prepare(k, q, v, mask, Wk, Wq, Wv, Wp):
    """Build (cached) the SPMD program and the 8 per-core input maps."""
    k = np.asarray(k, np.float32)
    q = np.asarray(q, np.float32)
    v = np.asarray(v, np.float32)
    Wk = np.asarray(Wk, np.float32)
    Wq = np.asarray(Wq, np.float32)
    Wv = np.asarray(Wv, np.float32)
    Wp = np.asarray(Wp, np.float32)

    nc, patterns = _get_program(mask)
    patflat = np.ascontiguousarray(patterns.reshape(-1, 128))

    def tr(x, npdt=NPBF16):  # [T, E] f32 -> [E, T] contiguous
        return np.ascontiguousarray(x.astype(npdt).T)

    def wcat(W, half, npdt=NPBF16):  # [H, E, D] -> [E, 512] for half's 8 heads
        return np.ascontiguousarray(
            W[half * HPC : (half + 1) * HPC].transpose(1, 0, 2).reshape(E, DC)
        ).astype(npdt)

    in_maps = []
    for c in range(8):
        b, half = divmod(c, 2)
        off = half * DC
        in_maps.append(
            {
                "qT": tr(q[b], NPF8),
                "kT": tr(k[b], NPF8),
                "vT": tr(v[b]),
                "wq": wcat(Wq, half, NPF8),
                "wk": wcat(Wk, half, NPF8),
                "wv": wcat(Wv, half),
                "wpT": np.ascontiguousarray(Wp[:, off : off + DC].T).astype(NPBF16),
                "pat": patflat,
                "selbc": _SELBC,
            }
        )
    return nc, in_maps


def kernel(k, q, v, mask, Wk, Wq, Wv, Wp, bp):
    bp = np.asarray(bp, np.float32)
    nc, in_maps = _prepare(k, q, v, mask, Wk, Wq, Wv, Wp)
    res = run_bass_kernel_spmd(nc, in_maps, list(range(8)))
    out = np.empty((B, T, E), np.float32)
    for b in range(B):
        yt = res.results[2 * b]["yT"] + res.results[2 * b + 1]["yT"]
        out[b] = yt.T + bp[None, :]
    return out
